# revision 13
# baseline (speedup 1.0000x reference)
"""FISM scoring kernel for 8 Trainium2 NeuronCores (Bass/Tile).

Reference computation (per batch element b):
    user_history[b] = Gi[hist_matrix[user[b]]]          # [L, F] gather (dominant cost)
    target[b]       = Gj[item[b]]                       # [F]
    s[b]  = sum_l hist_mask[user[b], l] * user_history[b, l]   # masked sum  [F]
    logit = hist_lens[user[b]]^-0.5 * (s[b] . target[b]) + Bu[user[b]] + Bi[item[b]]
    scores[b] = sigmoid(logit)

Sharding: data-parallel over batch. Each of the 8 cores handles 512 users with
all tables (Gi/Gj/hist_*/Bi/Bu) replicated; the host concatenates results.

Per-core device plan (BLOC=512 users, L=200, F=256):
  stage 0: gather hist_matrix/hist_mask/hist_lens/Bu rows by `user`, Bi/Gj by
           `item` (one indirect DMA each); round-trip hist/mask row blocks
           through DRAM scratch so the main loop can load them in flat
           (user*L) row order.
  main loop (32 tiles, 16 users per tile): each SBUF tile [128, 25*256] holds
           3200 gathered Gi rows (8 partitions per user, 25 rows each).
           - indirect-gather Gi rows (GPSIMD/SWDGE)
           - stream the tile to the user_history output (SP HWDGE)
           - masked per-user sums via 25 PE matmuls with mask-scaled
             block-diagonal selection matrices -> PSUM [16, 256]
           - fused DVE mul+reduce against the 16 target rows -> d[u]
  tail: logits = rsqrt(lens)*d + Bu + Bi, sigmoid via exp+reciprocal, write
        scores.
"""

import os
from contextlib import ExitStack

import numpy as np

import concourse.bacc as bacc
import concourse.bass as bass
import concourse.mybir as mybir
import concourse.tile as tile
from concourse.bass_utils import run_bass_kernel_spmd

NUM_USERS = 50000
NUM_ITEMS = 100000
L = 200
F = 256
B = 4096
NCORES = 8
BLOC = B // NCORES          # 512 users per core
UTILE = 16                  # users per main-loop tile
PPU = 128 // UTILE          # 8 partitions per user
RPU = L // PPU              # 25 history rows per partition
NT = BLOC // UTILE          # 32 main-loop tiles
ROWS_PER_TILE = 128 * RPU   # 3200 gathered rows per tile

f32 = mybir.dt.float32
i32 = mybir.dt.int32


def _body(ctx, tc, nc, t):
    """Emit the per-core program. `t` maps tensor name -> DRAM AP."""
    cut = os.environ.get("FISM_CUT", "none")  # none|tail|compute
    user, item = t["user"], t["item"]
    hist, lens, mask = t["hist_matrix"], t["hist_lens"], t["hist_mask"]
    Bi, Bu, Gi, Gj = t["Bi"], t["Bu"], t["Gi"], t["Gj"]
    scores_o, ub_o, ib_o = t["scores"], t["user_bias"], t["item_bias"]
    uh_o, tgt_o = t["user_history"], t["target"]

    const = ctx.enter_context(tc.tile_pool(name="const", bufs=1))
    setup = ctx.enter_context(tc.tile_pool(name="setup", bufs=1))
    small = ctx.enter_context(tc.tile_pool(name="small", bufs=3))
    big = ctx.enter_context(tc.tile_pool(name="big", bufs=3))
    psump = ctx.enter_context(tc.tile_pool(name="psump", bufs=2, space="PSUM"))
    dram = ctx.enter_context(tc.tile_pool(name="dram", bufs=1, space="DRAM"))

    # block-diagonal indicator: ind[p, m] = 1.0 iff p // PPU == m
    ind_np = np.zeros((128, UTILE), np.float32)
    ind_np[np.arange(128), np.arange(128) // PPU] = 1.0
    ind_dram = nc.inline_tensor(ind_np, name="ind_const").ap()
    ind_sb = const.tile([128, UTILE], f32)
    nc.sync.dma_start(out=ind_sb[:], in_=ind_dram[:, :])

    # ---- stage 0: per-user/per-item gathers ----
    user_sb = setup.tile([128, BLOC // 128], i32)
    nc.sync.dma_start(out=user_sb[:], in_=user.rearrange("(p c) -> p c", p=128))
    item_sb = setup.tile([128, BLOC // 128], i32)
    nc.sync.dma_start(out=item_sb[:], in_=item.rearrange("(p c) -> p c", p=128))

    # HW indirect DMA takes exactly one index per partition: issue one
    # gather per index column.
    C = BLOC // 128  # 4 columns of 128 users/items
    hist_sb = setup.tile([128, C * L], i32)
    mask_sb = setup.tile([128, C * L], f32)
    lens_sb = setup.tile([128, C], f32)
    bu_sb = setup.tile([128, C], f32)
    bi_sb = setup.tile([128, C], f32)
    tgt_sb = setup.tile([128, C * F], f32)
    for c in range(C):
        ui = user_sb[:, c:c + 1]
        ii = item_sb[:, c:c + 1]
        nc.gpsimd.indirect_dma_start(
            out=hist_sb[:, c * L:(c + 1) * L], out_offset=None, in_=hist[:, :],
            in_offset=bass.IndirectOffsetOnAxis(ap=ui, axis=0))
        nc.gpsimd.indirect_dma_start(
            out=mask_sb[:, c * L:(c + 1) * L], out_offset=None, in_=mask[:, :],
            in_offset=bass.IndirectOffsetOnAxis(ap=ui, axis=0))
        nc.gpsimd.indirect_dma_start(
            out=lens_sb[:, c:c + 1], out_offset=None, in_=lens.unsqueeze(1),
            in_offset=bass.IndirectOffsetOnAxis(ap=ui, axis=0))
        nc.gpsimd.indirect_dma_start(
            out=bu_sb[:, c:c + 1], out_offset=None, in_=Bu.unsqueeze(1),
            in_offset=bass.IndirectOffsetOnAxis(ap=ui, axis=0))
        nc.gpsimd.indirect_dma_start(
            out=bi_sb[:, c:c + 1], out_offset=None, in_=Bi.unsqueeze(1),
            in_offset=bass.IndirectOffsetOnAxis(ap=ii, axis=0))
        nc.gpsimd.indirect_dma_start(
            out=tgt_sb[:, c * F:(c + 1) * F], out_offset=None, in_=Gj[:, :],
            in_offset=bass.IndirectOffsetOnAxis(ap=ii, axis=0))

    # bias / target outputs are plain copies of the gathered rows
    nc.sync.dma_start(out=ub_o.rearrange("(p c) -> p c", p=128), in_=bu_sb[:])
    nc.sync.dma_start(out=ib_o.rearrange("(p c) -> p c", p=128), in_=bi_sb[:])
    nc.sync.dma_start(
        out=tgt_o.rearrange("(p c) f -> p (c f)", p=128), in_=tgt_sb[:])

    # round-trip hist/mask/target through DRAM scratch so the main loop can
    # re-load them in flat (user*L) row order / per-tile user order
    hist_flat = dram.tile([BLOC * L], i32)
    nc.sync.dma_start(out=hist_flat.rearrange("(p c) -> p c", p=128), in_=hist_sb[:])
    mask_flat = dram.tile([BLOC * L], f32)
    nc.sync.dma_start(out=mask_flat.rearrange("(p c) -> p c", p=128), in_=mask_sb[:])
    tgt_flat = dram.tile([BLOC, F], f32)
    nc.sync.dma_start(
        out=tgt_flat.rearrange("(p c) f -> p (c f)", p=128), in_=tgt_sb[:])

    # rsqrt(lens): DVE reciprocal (accurate) + ACT sqrt
    coeff_sb = setup.tile([128, BLOC // 128], f32)
    nc.vector.reciprocal(out=coeff_sb[:], in_=lens_sb[:])
    nc.scalar.sqrt(out=coeff_sb[:], in_=coeff_sb[:])

    # per-user dot-product accumulator, one column per main-loop tile
    dall = setup.tile([UTILE, NT], f32)

    # ---- main loop ----
    for ti in range(NT):
        r0 = ti * ROWS_PER_TILE
        idx_t = small.tile([128, RPU], i32)
        nc.scalar.dma_start(
            out=idx_t[:],
            in_=hist_flat[r0:r0 + ROWS_PER_TILE].rearrange("(p c) -> p c", p=128))
        mask_t = small.tile([128, RPU], f32)
        nc.scalar.dma_start(
            out=mask_t[:],
            in_=mask_flat[r0:r0 + ROWS_PER_TILE].rearrange("(p c) -> p c", p=128))
        tgt16 = small.tile([UTILE, F], f32)
        nc.scalar.dma_start(out=tgt16[:], in_=tgt_flat[ti * UTILE:(ti + 1) * UTILE, :])

        gath = big.tile([128, RPU * F], f32)
        for j in range(RPU):
            nc.gpsimd.indirect_dma_start(
                out=gath[:, j * F:(j + 1) * F], out_offset=None, in_=Gi[:, :],
                in_offset=bass.IndirectOffsetOnAxis(ap=idx_t[:, j:j + 1], axis=0))

        nc.sync.dma_start(
            out=uh_o[r0:r0 + ROWS_PER_TILE, :].rearrange(
                "(p r) f -> p (r f)", p=128),
            in_=gath[:])

        if cut == "compute":
            continue
        # sel[p, j*16+m] = mask_t[p, j] * ind[p, m]
        sel = small.tile([128, RPU * UTILE], f32)
        nc.vector.tensor_tensor(
            out=sel[:].rearrange("p (a b) -> p a b", b=UTILE),
            in0=mask_t[:].unsqueeze(2).to_broadcast([128, RPU, UTILE]),
            in1=ind_sb[:].unsqueeze(1).to_broadcast([128, RPU, UTILE]),
            op=mybir.AluOpType.mult)

        if cut == "sel":
            continue
        # PSUM[m, f] = sum_j sum_p sel[p, j*16+m] * gath[p, j*256+f]
        #            = masked sum over this tile's 16 users' history rows
        psum = psump.tile([UTILE, F], f32)
        for j in range(RPU):
            nc.tensor.matmul(
                out=psum[:],
                lhsT=sel[:, j * UTILE:(j + 1) * UTILE],
                rhs=gath[:, j * F:(j + 1) * F],
                start=(j == 0), stop=(j == RPU - 1))

        if cut == "ttr":
            continue
        prod16 = small.tile([UTILE, F], f32)
        nc.vector.tensor_tensor(out=prod16[:], in0=psum[:], in1=tgt16[:],
                                op=mybir.AluOpType.mult)
        nc.vector.tensor_reduce(out=dall[:, ti:ti + 1], in_=prod16[:],
                                axis=mybir.AxisListType.X,
                                op=mybir.AluOpType.add)

    # ---- tail: scores ----
    if cut in ("tail", "compute", "sel", "ttr"):
        nc.sync.dma_start(out=scores_o.rearrange("(p c) -> p c", p=128),
                          in_=coeff_sb[:])
        return
    d_flat = dram.tile([BLOC], f32)
    nc.sync.dma_start(
        out=d_flat.rearrange("(t m) -> m t", m=UTILE).unsqueeze(2),
        in_=dall[:].unsqueeze(2))
    d_sb = setup.tile([128, BLOC // 128], f32)
    nc.sync.dma_start(out=d_sb[:], in_=d_flat.rearrange("(p c) -> p c", p=128))

    logit = setup.tile([128, BLOC // 128], f32)
    nc.vector.tensor_tensor(out=logit[:], in0=d_sb[:], in1=coeff_sb[:],
                            op=mybir.AluOpType.mult)
    nc.vector.tensor_tensor(out=logit[:], in0=logit[:], in1=bu_sb[:],
                            op=mybir.AluOpType.add)
    nc.vector.tensor_tensor(out=logit[:], in0=logit[:], in1=bi_sb[:],
                            op=mybir.AluOpType.add)
    # sigmoid(x) = 1 / (1 + exp(-x)); Exp on ACT, accurate reciprocal on DVE
    expx = setup.tile([128, BLOC // 128], f32)
    nc.scalar.activation(out=expx[:], in_=logit[:],
                         func=mybir.ActivationFunctionType.Exp, scale=-1.0)
    nc.vector.tensor_scalar_add(out=expx[:], in0=expx[:], scalar1=1.0)
    scores_sb = setup.tile([128, BLOC // 128], f32)
    nc.vector.reciprocal(out=scores_sb[:], in_=expx[:])
    nc.sync.dma_start(out=scores_o.rearrange("(p c) -> p c", p=128),
                      in_=scores_sb[:])


def build():
    nc = bacc.Bacc("TRN2", target_bir_lowering=False, debug=False)
    t = {}
    t["user"] = nc.dram_tensor("user", [BLOC], i32, kind="ExternalInput").ap()
    t["item"] = nc.dram_tensor("item", [BLOC], i32, kind="ExternalInput").ap()
    t["hist_matrix"] = nc.dram_tensor(
        "hist_matrix", [NUM_USERS, L], i32, kind="ExternalInput").ap()
    t["hist_lens"] = nc.dram_tensor(
        "hist_lens", [NUM_USERS], f32, kind="ExternalInput").ap()
    t["hist_mask"] = nc.dram_tensor(
        "hist_mask", [NUM_USERS, L], f32, kind="ExternalInput").ap()
    t["Bi"] = nc.dram_tensor("Bi", [NUM_ITEMS], f32, kind="ExternalInput").ap()
    t["Bu"] = nc.dram_tensor("Bu", [NUM_USERS], f32, kind="ExternalInput").ap()
    t["Gi"] = nc.dram_tensor("Gi", [NUM_ITEMS, F], f32, kind="ExternalInput").ap()
    t["Gj"] = nc.dram_tensor("Gj", [NUM_ITEMS, F], f32, kind="ExternalInput").ap()
    t["scores"] = nc.dram_tensor("scores", [BLOC], f32, kind="ExternalOutput").ap()
    t["user_bias"] = nc.dram_tensor(
        "user_bias", [BLOC], f32, kind="ExternalOutput").ap()
    t["item_bias"] = nc.dram_tensor(
        "item_bias", [BLOC], f32, kind="ExternalOutput").ap()
    t["user_history"] = nc.dram_tensor(
        "user_history", [BLOC * L, F], f32, kind="ExternalOutput").ap()
    t["target"] = nc.dram_tensor("target", [BLOC, F], f32, kind="ExternalOutput").ap()

    with tile.TileContext(nc) as tc:
        with ExitStack() as ctx:
            _body(ctx, tc, nc, t)
    nc.compile()
    return nc


_NC = None
LAST_EXEC_TIME_NS = None


def kernel(**inputs):
    global _NC, LAST_EXEC_TIME_NS
    if _NC is None:
        _NC = build()
    nc = _NC

    user = np.asarray(inputs["user"])
    item = np.asarray(inputs["item"])
    shared = {
        "hist_matrix": np.ascontiguousarray(
            np.asarray(inputs["hist_matrix"], dtype=np.int32)),
        "hist_lens": np.ascontiguousarray(
            np.asarray(inputs["hist_lens"], dtype=np.float32)),
        "hist_mask": np.ascontiguousarray(
            np.asarray(inputs["hist_mask"], dtype=np.float32)),
        "Bi": np.ascontiguousarray(np.asarray(inputs["Bi"], dtype=np.float32)),
        "Bu": np.ascontiguousarray(np.asarray(inputs["Bu"], dtype=np.float32)),
        "Gi": np.ascontiguousarray(np.asarray(inputs["Gi"], dtype=np.float32)),
        "Gj": np.ascontiguousarray(np.asarray(inputs["Gj"], dtype=np.float32)),
    }
    in_maps = []
    for c in range(NCORES):
        sl = slice(c * BLOC, (c + 1) * BLOC)
        m = dict(shared)
        m["user"] = np.ascontiguousarray(user[sl].astype(np.int32, copy=False))
        m["item"] = np.ascontiguousarray(item[sl].astype(np.int32, copy=False))
        in_maps.append(m)

    res = run_bass_kernel_spmd(
        nc, in_maps, core_ids=list(range(NCORES)),
        trace=bool(int(os.environ.get("FISM_TRACE", "0"))))
    LAST_EXEC_TIME_NS = res.exec_time_ns
    rs = res.results

    scores = np.concatenate([rs[c]["scores"] for c in range(NCORES)])
    user_bias = np.concatenate([rs[c]["user_bias"] for c in range(NCORES)])
    item_bias = np.concatenate([rs[c]["item_bias"] for c in range(NCORES)])
    user_history = np.concatenate(
        [rs[c]["user_history"].reshape(BLOC, L, F) for c in range(NCORES)])
    target = np.concatenate([rs[c]["target"] for c in range(NCORES)])
    return (scores, user_bias, item_bias, user_history, target)


# revision 15
# speedup vs baseline: 7.7933x; 7.7933x over previous
"""FISM scoring kernel for 8 Trainium2 NeuronCores (Bass/Tile).

Reference computation (per batch element b):
    user_history[b] = Gi[hist_matrix[user[b]]]          # [L, F] gather (dominant cost)
    target[b]       = Gj[item[b]]                       # [F]
    s[b]  = sum_l hist_mask[user[b], l] * user_history[b, l]   # masked sum  [F]
    logit = hist_lens[user[b]]^-0.5 * (s[b] . target[b]) + Bu[user[b]] + Bi[item[b]]
    scores[b] = sigmoid(logit)

Sharding: data-parallel over batch. Each of the 8 cores handles 512 users with
all tables (Gi/Gj/hist_*/Bi/Bu) replicated; the host concatenates results.

Per-core device plan (BLOC=512 users, L=200, F=256):
  stage 0: gather hist_matrix/hist_mask/hist_lens/Bu rows by `user`, Bi/Gj by
           `item` (one indirect DMA each); round-trip hist/mask row blocks
           through DRAM scratch so the main loop can load them in flat
           (user*L) row order.
  main loop (32 tiles, 16 users per tile): each SBUF tile [128, 25*256] holds
           3200 gathered Gi rows (8 partitions per user, 25 rows each).
           - indirect-gather Gi rows (GPSIMD/SWDGE)
           - stream the tile to the user_history output (SP HWDGE)
           - masked per-user sums via 25 PE matmuls with mask-scaled
             block-diagonal selection matrices -> PSUM [16, 256]
           - fused DVE mul+reduce against the 16 target rows -> d[u]
  tail: logits = rsqrt(lens)*d + Bu + Bi, sigmoid via exp+reciprocal, write
        scores.
"""

import os
from contextlib import ExitStack

import numpy as np

import concourse.bacc as bacc
import concourse.bass as bass
import concourse.mybir as mybir
import concourse.tile as tile
from concourse.bass_utils import run_bass_kernel_spmd

NUM_USERS = 50000
NUM_ITEMS = 100000
L = 200
F = 256
B = 4096
NCORES = 8
BLOC = B // NCORES          # 512 users per core
UTILE = 16                  # users per main-loop tile
PPU = 128 // UTILE          # 8 partitions per user
RPU = L // PPU              # 25 history rows per partition
NT = BLOC // UTILE          # 32 main-loop tiles
ROWS_PER_TILE = 128 * RPU   # 3200 gathered rows per tile

f32 = mybir.dt.float32
i32 = mybir.dt.int32


def _body(ctx, tc, nc, t):
    """Emit the per-core program. `t` maps tensor name -> DRAM AP."""
    cut = os.environ.get("FISM_CUT", "none")  # none|tail|compute
    user, item = t["user"], t["item"]
    hist, lens, mask = t["hist_matrix"], t["hist_lens"], t["hist_mask"]
    Bi, Bu, Gi, Gj = t["Bi"], t["Bu"], t["Gi"], t["Gj"]
    scores_o, ub_o, ib_o = t["scores"], t["user_bias"], t["item_bias"]
    uh_o, tgt_o = t["user_history"], t["target"]

    const = ctx.enter_context(tc.tile_pool(name="const", bufs=1))
    setup = ctx.enter_context(tc.tile_pool(name="setup", bufs=1))
    small = ctx.enter_context(tc.tile_pool(name="small", bufs=3))
    big = ctx.enter_context(tc.tile_pool(name="big", bufs=3))
    psump = ctx.enter_context(tc.tile_pool(name="psump", bufs=2, space="PSUM"))
    dram = ctx.enter_context(tc.tile_pool(name="dram", bufs=1, space="DRAM"))

    # block-diagonal indicator: ind[p, m] = 1.0 iff p // PPU == m
    ind_np = np.zeros((128, UTILE), np.float32)
    ind_np[np.arange(128), np.arange(128) // PPU] = 1.0
    ind_dram = nc.inline_tensor(
        ind_np, name=f"ind_const_{nc.next_id()}").ap()
    ind_sb = const.tile([128, UTILE], f32)
    nc.sync.dma_start(out=ind_sb[:], in_=ind_dram[:, :])

    # ---- stage 0: per-user/per-item gathers ----
    user_sb = setup.tile([128, BLOC // 128], i32)
    nc.sync.dma_start(out=user_sb[:], in_=user.rearrange("(p c) -> p c", p=128))
    item_sb = setup.tile([128, BLOC // 128], i32)
    nc.sync.dma_start(out=item_sb[:], in_=item.rearrange("(p c) -> p c", p=128))

    # HW indirect DMA takes exactly one index per partition: issue one
    # gather per index column.
    C = BLOC // 128  # 4 columns of 128 users/items
    hist_sb = setup.tile([128, C * L], i32)
    mask_sb = setup.tile([128, C * L], f32)
    lens_sb = setup.tile([128, C], f32)
    bu_sb = setup.tile([128, C], f32)
    bi_sb = setup.tile([128, C], f32)
    tgt_sb = setup.tile([128, C * F], f32)
    for c in range(C):
        ui = user_sb[:, c:c + 1]
        ii = item_sb[:, c:c + 1]
        nc.gpsimd.indirect_dma_start(
            out=hist_sb[:, c * L:(c + 1) * L], out_offset=None, in_=hist[:, :],
            in_offset=bass.IndirectOffsetOnAxis(ap=ui, axis=0))
        nc.gpsimd.indirect_dma_start(
            out=mask_sb[:, c * L:(c + 1) * L], out_offset=None, in_=mask[:, :],
            in_offset=bass.IndirectOffsetOnAxis(ap=ui, axis=0))
        nc.gpsimd.indirect_dma_start(
            out=lens_sb[:, c:c + 1], out_offset=None, in_=lens.unsqueeze(1),
            in_offset=bass.IndirectOffsetOnAxis(ap=ui, axis=0))
        nc.gpsimd.indirect_dma_start(
            out=bu_sb[:, c:c + 1], out_offset=None, in_=Bu.unsqueeze(1),
            in_offset=bass.IndirectOffsetOnAxis(ap=ui, axis=0))
        nc.gpsimd.indirect_dma_start(
            out=bi_sb[:, c:c + 1], out_offset=None, in_=Bi.unsqueeze(1),
            in_offset=bass.IndirectOffsetOnAxis(ap=ii, axis=0))
        nc.gpsimd.indirect_dma_start(
            out=tgt_sb[:, c * F:(c + 1) * F], out_offset=None, in_=Gj[:, :],
            in_offset=bass.IndirectOffsetOnAxis(ap=ii, axis=0))

    # bias / target outputs are plain copies of the gathered rows
    nc.sync.dma_start(out=ub_o.rearrange("(p c) -> p c", p=128), in_=bu_sb[:])
    nc.sync.dma_start(out=ib_o.rearrange("(p c) -> p c", p=128), in_=bi_sb[:])
    nc.sync.dma_start(
        out=tgt_o.rearrange("(p c) f -> p (c f)", p=128), in_=tgt_sb[:])

    # round-trip hist/mask/target through DRAM scratch so the main loop can
    # re-load them in flat (user*L) row order / per-tile user order
    hist_flat = dram.tile([BLOC * L], i32)
    nc.sync.dma_start(out=hist_flat.rearrange("(p c) -> p c", p=128), in_=hist_sb[:])
    mask_flat = dram.tile([BLOC * L], f32)
    nc.sync.dma_start(out=mask_flat.rearrange("(p c) -> p c", p=128), in_=mask_sb[:])
    tgt_flat = dram.tile([BLOC, F], f32)
    nc.sync.dma_start(
        out=tgt_flat.rearrange("(p c) f -> p (c f)", p=128), in_=tgt_sb[:])

    # rsqrt(lens): DVE reciprocal (accurate) + ACT sqrt
    coeff_sb = setup.tile([128, BLOC // 128], f32)
    nc.vector.reciprocal(out=coeff_sb[:], in_=lens_sb[:])
    nc.scalar.sqrt(out=coeff_sb[:], in_=coeff_sb[:])

    # per-user dot-product accumulator, one column per main-loop tile
    dall = setup.tile([UTILE, NT], f32)

    # ---- main loop ----
    for ti in range(NT):
        r0 = ti * ROWS_PER_TILE
        idx_t = small.tile([128, RPU], i32)
        nc.scalar.dma_start(
            out=idx_t[:],
            in_=hist_flat[r0:r0 + ROWS_PER_TILE].rearrange("(p c) -> p c", p=128))
        mask_t = small.tile([128, RPU], f32)
        nc.scalar.dma_start(
            out=mask_t[:],
            in_=mask_flat[r0:r0 + ROWS_PER_TILE].rearrange("(p c) -> p c", p=128))
        tgt16 = small.tile([UTILE, F], f32)
        nc.scalar.dma_start(out=tgt16[:], in_=tgt_flat[ti * UTILE:(ti + 1) * UTILE, :])

        gath = big.tile([128, RPU * F], f32)
        for j in range(RPU):
            nc.gpsimd.indirect_dma_start(
                out=gath[:, j * F:(j + 1) * F], out_offset=None, in_=Gi[:, :],
                in_offset=bass.IndirectOffsetOnAxis(ap=idx_t[:, j:j + 1], axis=0))

        nc.sync.dma_start(
            out=uh_o[r0:r0 + ROWS_PER_TILE, :].rearrange(
                "(p r) f -> p (r f)", p=128),
            in_=gath[:])

        if cut == "compute":
            continue
        # sel[p, j*16+m] = mask_t[p, j] * ind[p, m]
        sel = small.tile([128, RPU * UTILE], f32)
        nc.vector.tensor_tensor(
            out=sel[:].rearrange("p (a b) -> p a b", b=UTILE),
            in0=mask_t[:].unsqueeze(2).to_broadcast([128, RPU, UTILE]),
            in1=ind_sb[:].unsqueeze(1).to_broadcast([128, RPU, UTILE]),
            op=mybir.AluOpType.mult)

        if cut == "sel":
            continue
        # PSUM[m, f] = sum_j sum_p sel[p, j*16+m] * gath[p, j*256+f]
        #            = masked sum over this tile's 16 users' history rows
        psum = psump.tile([UTILE, F], f32)
        for j in range(RPU):
            nc.tensor.matmul(
                out=psum[:],
                lhsT=sel[:, j * UTILE:(j + 1) * UTILE],
                rhs=gath[:, j * F:(j + 1) * F],
                start=(j == 0), stop=(j == RPU - 1))

        if cut == "ttr":
            continue
        prod16 = small.tile([UTILE, F], f32)
        nc.vector.tensor_tensor(out=prod16[:], in0=psum[:], in1=tgt16[:],
                                op=mybir.AluOpType.mult)
        nc.vector.tensor_reduce(out=dall[:, ti:ti + 1], in_=prod16[:],
                                axis=mybir.AxisListType.X,
                                op=mybir.AluOpType.add)

    # ---- tail: scores ----
    if cut in ("tail", "compute", "sel", "ttr"):
        nc.sync.dma_start(out=scores_o.rearrange("(p c) -> p c", p=128),
                          in_=coeff_sb[:])
        return
    d_flat = dram.tile([BLOC], f32)
    nc.sync.dma_start(
        out=d_flat.rearrange("(t m) -> m t", m=UTILE).unsqueeze(2),
        in_=dall[:].unsqueeze(2))
    d_sb = setup.tile([128, BLOC // 128], f32)
    nc.sync.dma_start(out=d_sb[:], in_=d_flat.rearrange("(p c) -> p c", p=128))

    logit = setup.tile([128, BLOC // 128], f32)
    nc.vector.tensor_tensor(out=logit[:], in0=d_sb[:], in1=coeff_sb[:],
                            op=mybir.AluOpType.mult)
    nc.vector.tensor_tensor(out=logit[:], in0=logit[:], in1=bu_sb[:],
                            op=mybir.AluOpType.add)
    nc.vector.tensor_tensor(out=logit[:], in0=logit[:], in1=bi_sb[:],
                            op=mybir.AluOpType.add)
    # sigmoid(x) = 1 / (1 + exp(-x)); Exp on ACT, accurate reciprocal on DVE
    expx = setup.tile([128, BLOC // 128], f32)
    nc.scalar.activation(out=expx[:], in_=logit[:],
                         func=mybir.ActivationFunctionType.Exp, scale=-1.0)
    nc.vector.tensor_scalar_add(out=expx[:], in0=expx[:], scalar1=1.0)
    scores_sb = setup.tile([128, BLOC // 128], f32)
    nc.vector.reciprocal(out=scores_sb[:], in_=expx[:])
    nc.sync.dma_start(out=scores_o.rearrange("(p c) -> p c", p=128),
                      in_=scores_sb[:])


def build():
    nc = bacc.Bacc("TRN2", target_bir_lowering=False, debug=False)
    t = {}
    t["user"] = nc.dram_tensor("user", [BLOC], i32, kind="ExternalInput").ap()
    t["item"] = nc.dram_tensor("item", [BLOC], i32, kind="ExternalInput").ap()
    t["hist_matrix"] = nc.dram_tensor(
        "hist_matrix", [NUM_USERS, L], i32, kind="ExternalInput").ap()
    t["hist_lens"] = nc.dram_tensor(
        "hist_lens", [NUM_USERS], f32, kind="ExternalInput").ap()
    t["hist_mask"] = nc.dram_tensor(
        "hist_mask", [NUM_USERS, L], f32, kind="ExternalInput").ap()
    t["Bi"] = nc.dram_tensor("Bi", [NUM_ITEMS], f32, kind="ExternalInput").ap()
    t["Bu"] = nc.dram_tensor("Bu", [NUM_USERS], f32, kind="ExternalInput").ap()
    t["Gi"] = nc.dram_tensor("Gi", [NUM_ITEMS, F], f32, kind="ExternalInput").ap()
    t["Gj"] = nc.dram_tensor("Gj", [NUM_ITEMS, F], f32, kind="ExternalInput").ap()
    t["scores"] = nc.dram_tensor("scores", [BLOC], f32, kind="ExternalOutput").ap()
    t["user_bias"] = nc.dram_tensor(
        "user_bias", [BLOC], f32, kind="ExternalOutput").ap()
    t["item_bias"] = nc.dram_tensor(
        "item_bias", [BLOC], f32, kind="ExternalOutput").ap()
    t["user_history"] = nc.dram_tensor(
        "user_history", [BLOC * L, F], f32, kind="ExternalOutput").ap()
    t["target"] = nc.dram_tensor("target", [BLOC, F], f32, kind="ExternalOutput").ap()

    with tile.TileContext(nc) as tc:
        for _ in range(int(os.environ.get("FISM_REPS", "1"))):
            with ExitStack() as ctx:
                _body(ctx, tc, nc, t)
    nc.compile()
    return nc


_NC = None
LAST_EXEC_TIME_NS = None


def kernel(**inputs):
    global _NC, LAST_EXEC_TIME_NS
    if _NC is None:
        _NC = build()
    nc = _NC

    user = np.asarray(inputs["user"])
    item = np.asarray(inputs["item"])
    shared = {
        "hist_matrix": np.ascontiguousarray(
            np.asarray(inputs["hist_matrix"], dtype=np.int32)),
        "hist_lens": np.ascontiguousarray(
            np.asarray(inputs["hist_lens"], dtype=np.float32)),
        "hist_mask": np.ascontiguousarray(
            np.asarray(inputs["hist_mask"], dtype=np.float32)),
        "Bi": np.ascontiguousarray(np.asarray(inputs["Bi"], dtype=np.float32)),
        "Bu": np.ascontiguousarray(np.asarray(inputs["Bu"], dtype=np.float32)),
        "Gi": np.ascontiguousarray(np.asarray(inputs["Gi"], dtype=np.float32)),
        "Gj": np.ascontiguousarray(np.asarray(inputs["Gj"], dtype=np.float32)),
    }
    in_maps = []
    for c in range(NCORES):
        sl = slice(c * BLOC, (c + 1) * BLOC)
        m = dict(shared)
        m["user"] = np.ascontiguousarray(user[sl].astype(np.int32, copy=False))
        m["item"] = np.ascontiguousarray(item[sl].astype(np.int32, copy=False))
        in_maps.append(m)

    res = run_bass_kernel_spmd(
        nc, in_maps, core_ids=list(range(NCORES)),
        trace=bool(int(os.environ.get("FISM_TRACE", "0"))))
    LAST_EXEC_TIME_NS = res.exec_time_ns
    rs = res.results

    scores = np.concatenate([rs[c]["scores"] for c in range(NCORES)])
    user_bias = np.concatenate([rs[c]["user_bias"] for c in range(NCORES)])
    item_bias = np.concatenate([rs[c]["item_bias"] for c in range(NCORES)])
    user_history = np.concatenate(
        [rs[c]["user_history"].reshape(BLOC, L, F) for c in range(NCORES)])
    target = np.concatenate([rs[c]["target"] for c in range(NCORES)])
    return (scores, user_bias, item_bias, user_history, target)


# revision 17
# speedup vs baseline: 12.9224x; 1.6581x over previous
"""FISM scoring kernel for 8 Trainium2 NeuronCores (Bass/Tile).

Reference computation (per batch element b):
    user_history[b] = Gi[hist_matrix[user[b]]]          # [L, F] gather (dominant cost)
    target[b]       = Gj[item[b]]                       # [F]
    s[b]  = sum_l hist_mask[user[b], l] * user_history[b, l]   # masked sum  [F]
    logit = hist_lens[user[b]]^-0.5 * (s[b] . target[b]) + Bu[user[b]] + Bi[item[b]]
    scores[b] = sigmoid(logit)

Sharding: data-parallel over batch. Each of the 8 cores handles 512 users with
all tables (Gi/Gj/hist_*/Bi/Bu) replicated; the host concatenates results.

Per-core device plan (BLOC=512 users, L=200, F=256):
  stage 0: gather hist_matrix/hist_mask/hist_lens/Bu rows by `user`, Bi/Gj by
           `item` (one indirect DMA each); round-trip hist/mask row blocks
           through DRAM scratch so the main loop can load them in flat
           (user*L) row order.
  main loop (32 tiles, 16 users per tile): each SBUF tile [128, 25*256] holds
           3200 gathered Gi rows (8 partitions per user, 25 rows each).
           - indirect-gather Gi rows (GPSIMD/SWDGE)
           - stream the tile to the user_history output (SP HWDGE)
           - masked per-user sums via 25 PE matmuls with mask-scaled
             block-diagonal selection matrices -> PSUM [16, 256]
           - fused DVE mul+reduce against the 16 target rows -> d[u]
  tail: logits = rsqrt(lens)*d + Bu + Bi, sigmoid via exp+reciprocal, write
        scores.
"""

import os
from contextlib import ExitStack

import numpy as np

import concourse.bacc as bacc
import concourse.bass as bass
import concourse.mybir as mybir
import concourse.tile as tile
from concourse.bass_utils import run_bass_kernel_spmd

NUM_USERS = 50000
NUM_ITEMS = 100000
L = 200
F = 256
B = 4096
NCORES = 8
BLOC = B // NCORES          # 512 users per core
UTILE = 16                  # users per main-loop tile
PPU = 128 // UTILE          # 8 partitions per user
RPU = L // PPU              # 25 history rows per partition
NT = BLOC // UTILE          # 32 main-loop tiles
ROWS_PER_TILE = 128 * RPU   # 3200 gathered rows per tile

f32 = mybir.dt.float32
i32 = mybir.dt.int32


def _body(ctx, tc, nc, t):
    """Emit the per-core program. `t` maps tensor name -> DRAM AP."""
    cut = os.environ.get("FISM_CUT", "none")  # none|tail|compute
    user, item = t["user"], t["item"]
    hist, lens, mask = t["hist_matrix"], t["hist_lens"], t["hist_mask"]
    Bi, Bu, Gi, Gj = t["Bi"], t["Bu"], t["Gi"], t["Gj"]
    scores_o, ub_o, ib_o = t["scores"], t["user_bias"], t["item_bias"]
    uh_o, tgt_o = t["user_history"], t["target"]

    const = ctx.enter_context(tc.tile_pool(name="const", bufs=1))
    setup = ctx.enter_context(tc.tile_pool(name="setup", bufs=1))
    small = ctx.enter_context(tc.tile_pool(name="small", bufs=3))
    big = ctx.enter_context(tc.tile_pool(name="big", bufs=3))
    psump = ctx.enter_context(tc.tile_pool(name="psump", bufs=2, space="PSUM"))
    dram = ctx.enter_context(tc.tile_pool(name="dram", bufs=1, space="DRAM"))

    # block-diagonal indicator: ind[p, m] = 1.0 iff p // PPU == m
    ind_np = np.zeros((128, UTILE), np.float32)
    ind_np[np.arange(128), np.arange(128) // PPU] = 1.0
    ind_dram = nc.inline_tensor(
        ind_np, name=f"ind_const_{nc.next_id()}").ap()
    ind_sb = const.tile([128, UTILE], f32)
    nc.sync.dma_start(out=ind_sb[:], in_=ind_dram[:, :])

    # ---- stage 0: per-user/per-item gathers ----
    user_sb = setup.tile([128, BLOC // 128], i32)
    nc.sync.dma_start(out=user_sb[:], in_=user.rearrange("(p c) -> p c", p=128))
    item_sb = setup.tile([128, BLOC // 128], i32)
    nc.sync.dma_start(out=item_sb[:], in_=item.rearrange("(p c) -> p c", p=128))

    # HW indirect DMA takes exactly one index per partition: issue one
    # gather per index column.
    C = BLOC // 128  # 4 columns of 128 users/items
    hist_sb = setup.tile([128, C * L], i32)
    mask_sb = setup.tile([128, C * L], f32)
    lens_sb = setup.tile([128, C], f32)
    bu_sb = setup.tile([128, C], f32)
    bi_sb = setup.tile([128, C], f32)
    tgt_sb = setup.tile([128, C * F], f32)
    for c in range(C):
        ui = user_sb[:, c:c + 1]
        ii = item_sb[:, c:c + 1]
        nc.gpsimd.indirect_dma_start(
            out=hist_sb[:, c * L:(c + 1) * L], out_offset=None, in_=hist[:, :],
            in_offset=bass.IndirectOffsetOnAxis(ap=ui, axis=0))
        nc.gpsimd.indirect_dma_start(
            out=mask_sb[:, c * L:(c + 1) * L], out_offset=None, in_=mask[:, :],
            in_offset=bass.IndirectOffsetOnAxis(ap=ui, axis=0))
        nc.gpsimd.indirect_dma_start(
            out=lens_sb[:, c:c + 1], out_offset=None, in_=lens.unsqueeze(1),
            in_offset=bass.IndirectOffsetOnAxis(ap=ui, axis=0))
        nc.gpsimd.indirect_dma_start(
            out=bu_sb[:, c:c + 1], out_offset=None, in_=Bu.unsqueeze(1),
            in_offset=bass.IndirectOffsetOnAxis(ap=ui, axis=0))
        nc.gpsimd.indirect_dma_start(
            out=bi_sb[:, c:c + 1], out_offset=None, in_=Bi.unsqueeze(1),
            in_offset=bass.IndirectOffsetOnAxis(ap=ii, axis=0))
        nc.gpsimd.indirect_dma_start(
            out=tgt_sb[:, c * F:(c + 1) * F], out_offset=None, in_=Gj[:, :],
            in_offset=bass.IndirectOffsetOnAxis(ap=ii, axis=0))

    # bias / target outputs are plain copies of the gathered rows
    nc.sync.dma_start(out=ub_o.rearrange("(p c) -> p c", p=128), in_=bu_sb[:])
    nc.sync.dma_start(out=ib_o.rearrange("(p c) -> p c", p=128), in_=bi_sb[:])
    nc.sync.dma_start(
        out=tgt_o.rearrange("(p c) f -> p (c f)", p=128), in_=tgt_sb[:])

    # round-trip hist/mask/target through DRAM scratch so the main loop can
    # re-load them in flat (user*L) row order / per-tile user order
    hist_flat = dram.tile([BLOC * L], i32)
    nc.sync.dma_start(out=hist_flat.rearrange("(p c) -> p c", p=128), in_=hist_sb[:])
    mask_flat = dram.tile([BLOC * L], f32)
    nc.sync.dma_start(out=mask_flat.rearrange("(p c) -> p c", p=128), in_=mask_sb[:])
    tgt_flat = dram.tile([BLOC, F], f32)
    nc.sync.dma_start(
        out=tgt_flat.rearrange("(p c) f -> p (c f)", p=128), in_=tgt_sb[:])

    # rsqrt(lens): DVE reciprocal (accurate) + ACT sqrt
    coeff_sb = setup.tile([128, BLOC // 128], f32)
    nc.vector.reciprocal(out=coeff_sb[:], in_=lens_sb[:])
    nc.scalar.sqrt(out=coeff_sb[:], in_=coeff_sb[:])

    # per-user dot-product accumulator, one column per main-loop tile
    dall = setup.tile([UTILE, NT], f32)

    # ---- main loop ----
    for ti in range(NT):
        r0 = ti * ROWS_PER_TILE
        idx_t = small.tile([128, RPU], i32)
        nc.scalar.dma_start(
            out=idx_t[:],
            in_=hist_flat[r0:r0 + ROWS_PER_TILE].rearrange("(p c) -> p c", p=128))
        mask_t = small.tile([128, RPU], f32)
        nc.scalar.dma_start(
            out=mask_t[:],
            in_=mask_flat[r0:r0 + ROWS_PER_TILE].rearrange("(p c) -> p c", p=128))
        tgt16 = small.tile([UTILE, F], f32)
        nc.scalar.dma_start(out=tgt16[:], in_=tgt_flat[ti * UTILE:(ti + 1) * UTILE, :])

        gath = big.tile([128, RPU * F], f32)
        for j in range(RPU):
            nc.gpsimd.indirect_dma_start(
                out=gath[:, j * F:(j + 1) * F], out_offset=None, in_=Gi[:, :],
                in_offset=bass.IndirectOffsetOnAxis(ap=idx_t[:, j:j + 1], axis=0))

        nc.sync.dma_start(
            out=uh_o[r0:r0 + ROWS_PER_TILE, :].rearrange(
                "(p r) f -> p (r f)", p=128),
            in_=gath[:])

        if cut == "compute":
            continue
        # sel[p, j*16+m] = mask_t[p, j] * ind[p, m]
        sel = small.tile([128, RPU * UTILE], f32)
        nc.vector.tensor_tensor(
            out=sel[:].rearrange("p (a b) -> p a b", b=UTILE),
            in0=mask_t[:].unsqueeze(2).to_broadcast([128, RPU, UTILE]),
            in1=ind_sb[:].unsqueeze(1).to_broadcast([128, RPU, UTILE]),
            op=mybir.AluOpType.mult)

        if cut == "sel":
            continue
        # PSUM[m, f] = sum_j sum_p sel[p, j*16+m] * gath[p, j*256+f]
        #            = masked sum over this tile's 16 users' history rows
        psum = psump.tile([UTILE, F], f32)
        for j in range(RPU):
            nc.tensor.matmul(
                out=psum[:],
                lhsT=sel[:, j * UTILE:(j + 1) * UTILE],
                rhs=gath[:, j * F:(j + 1) * F],
                start=(j == 0), stop=(j == RPU - 1))

        if cut == "ttr":
            continue
        prod16 = small.tile([UTILE, F], f32)
        nc.vector.tensor_tensor(out=prod16[:], in0=psum[:], in1=tgt16[:],
                                op=mybir.AluOpType.mult)
        nc.vector.tensor_reduce(out=dall[:, ti:ti + 1], in_=prod16[:],
                                axis=mybir.AxisListType.X,
                                op=mybir.AluOpType.add)

    # ---- tail: scores ----
    if cut in ("tail", "compute", "sel", "ttr"):
        nc.sync.dma_start(out=scores_o.rearrange("(p c) -> p c", p=128),
                          in_=coeff_sb[:])
        return
    d_flat = dram.tile([BLOC], f32)
    nc.sync.dma_start(
        out=d_flat.rearrange("(t m) -> m t", m=UTILE).unsqueeze(2),
        in_=dall[:].unsqueeze(2))
    d_sb = setup.tile([128, BLOC // 128], f32)
    nc.sync.dma_start(out=d_sb[:], in_=d_flat.rearrange("(p c) -> p c", p=128))

    logit = setup.tile([128, BLOC // 128], f32)
    nc.vector.tensor_tensor(out=logit[:], in0=d_sb[:], in1=coeff_sb[:],
                            op=mybir.AluOpType.mult)
    nc.vector.tensor_tensor(out=logit[:], in0=logit[:], in1=bu_sb[:],
                            op=mybir.AluOpType.add)
    nc.vector.tensor_tensor(out=logit[:], in0=logit[:], in1=bi_sb[:],
                            op=mybir.AluOpType.add)
    # sigmoid(x) = 1 / (1 + exp(-x)); Exp on ACT, accurate reciprocal on DVE
    expx = setup.tile([128, BLOC // 128], f32)
    nc.scalar.activation(out=expx[:], in_=logit[:],
                         func=mybir.ActivationFunctionType.Exp, scale=-1.0)
    nc.vector.tensor_scalar_add(out=expx[:], in0=expx[:], scalar1=1.0)
    scores_sb = setup.tile([128, BLOC // 128], f32)
    nc.vector.reciprocal(out=scores_sb[:], in_=expx[:])
    nc.sync.dma_start(out=scores_o.rearrange("(p c) -> p c", p=128),
                      in_=scores_sb[:])


def build():
    nc = bacc.Bacc("TRN2", target_bir_lowering=False, debug=False)
    t = {}
    t["user"] = nc.dram_tensor("user", [BLOC], i32, kind="ExternalInput").ap()
    t["item"] = nc.dram_tensor("item", [BLOC], i32, kind="ExternalInput").ap()
    t["hist_matrix"] = nc.dram_tensor(
        "hist_matrix", [NUM_USERS, L], i32, kind="ExternalInput").ap()
    t["hist_lens"] = nc.dram_tensor(
        "hist_lens", [NUM_USERS], f32, kind="ExternalInput").ap()
    t["hist_mask"] = nc.dram_tensor(
        "hist_mask", [NUM_USERS, L], f32, kind="ExternalInput").ap()
    t["Bi"] = nc.dram_tensor("Bi", [NUM_ITEMS], f32, kind="ExternalInput").ap()
    t["Bu"] = nc.dram_tensor("Bu", [NUM_USERS], f32, kind="ExternalInput").ap()
    t["Gi"] = nc.dram_tensor("Gi", [NUM_ITEMS, F], f32, kind="ExternalInput").ap()
    t["Gj"] = nc.dram_tensor("Gj", [NUM_ITEMS, F], f32, kind="ExternalInput").ap()
    t["scores"] = nc.dram_tensor("scores", [BLOC], f32, kind="ExternalOutput").ap()
    t["user_bias"] = nc.dram_tensor(
        "user_bias", [BLOC], f32, kind="ExternalOutput").ap()
    t["item_bias"] = nc.dram_tensor(
        "item_bias", [BLOC], f32, kind="ExternalOutput").ap()
    t["user_history"] = nc.dram_tensor(
        "user_history", [BLOC * L, F], f32, kind="ExternalOutput").ap()
    t["target"] = nc.dram_tensor("target", [BLOC, F], f32, kind="ExternalOutput").ap()

    with tile.TileContext(nc) as tc:
        for _ in range(int(os.environ.get("FISM_REPS", "1"))):
            with ExitStack() as ctx:
                _body(ctx, tc, nc, t)
    nc.compile()
    return nc


_NC = None
LAST_EXEC_TIME_NS = None


def kernel(**inputs):
    global _NC, LAST_EXEC_TIME_NS
    if _NC is None:
        _NC = build()
    nc = _NC

    user = np.asarray(inputs["user"])
    item = np.asarray(inputs["item"])
    shared = {
        "hist_matrix": np.ascontiguousarray(
            np.asarray(inputs["hist_matrix"], dtype=np.int32)),
        "hist_lens": np.ascontiguousarray(
            np.asarray(inputs["hist_lens"], dtype=np.float32)),
        "hist_mask": np.ascontiguousarray(
            np.asarray(inputs["hist_mask"], dtype=np.float32)),
        "Bi": np.ascontiguousarray(np.asarray(inputs["Bi"], dtype=np.float32)),
        "Bu": np.ascontiguousarray(np.asarray(inputs["Bu"], dtype=np.float32)),
        "Gi": np.ascontiguousarray(np.asarray(inputs["Gi"], dtype=np.float32)),
        "Gj": np.ascontiguousarray(np.asarray(inputs["Gj"], dtype=np.float32)),
    }
    in_maps = []
    for c in range(NCORES):
        sl = slice(c * BLOC, (c + 1) * BLOC)
        m = dict(shared)
        m["user"] = np.ascontiguousarray(user[sl].astype(np.int32, copy=False))
        m["item"] = np.ascontiguousarray(item[sl].astype(np.int32, copy=False))
        in_maps.append(m)

    res = run_bass_kernel_spmd(
        nc, in_maps, core_ids=list(range(NCORES)),
        trace=bool(int(os.environ.get("FISM_TRACE", "0"))))
    LAST_EXEC_TIME_NS = res.exec_time_ns
    rs = res.results

    scores = np.concatenate([rs[c]["scores"] for c in range(NCORES)])
    user_bias = np.concatenate([rs[c]["user_bias"] for c in range(NCORES)])
    item_bias = np.concatenate([rs[c]["item_bias"] for c in range(NCORES)])
    user_history = np.concatenate(
        [rs[c]["user_history"].reshape(BLOC, L, F) for c in range(NCORES)])
    target = np.concatenate([rs[c]["target"] for c in range(NCORES)])
    return (scores, user_bias, item_bias, user_history, target)
